# revision 29
# baseline (speedup 1.0000x reference)
"""ConvolutionKAN Trainium2 kernel (8-core SPMD, data-parallel over batch).

Math: per conv patch element x (3x3x32 taps x channels) the reference takes a
cubic B-spline basis beta_0..7(x) on a uniform grid over [-1, 1], contracts
with (spline_kernel * scale_factor), and adds silu(x) @ scale_factor + bias.

Basis used on device ("mirror cube" basis): with t = 2.5 x + 2.5 in [0, 5),
the 8-dim spline space (incl. constants) is spanned by {1} and the 7 bounded-
conditioning features

    M_k = relu(k - t)^3  (k = 3, 2, 1),   R_m = relu(t - m)^3  (m = 1..4).

With F_m the third differences of relu^3 (bounded in [0,6]) one has
beta_j = (F_{j-3} - F_{j-2})/6, and expanding F in M/R gives exact small-
coefficient folded weights (see _prep_weights):

    out = sum_i [ w3 + M1*(-D0+3D1-3D2) + M2*(-D1+3D2) + M3*(-D2) + R1*D3
                  + R2*(-3D3+D4) + R3*(3D3-3D4+D5) + R4*(-D3+3D4-3D5+D6) ]
    D_k = (w_{k+1} - w_k)/6,  w_j = spline_kernel * scale_factor.

Unlike the truncated-power expansion (x, x^2, x^3, R...), this basis has NO
weight amplification (|W| <= 0.64 vs 13.9), so the whole matmul path runs in
bf16 (verified max rel err ~5e-3 vs tolerance 2e-2).  bf16 rhs streams at the
PE array limit (~207 ns per K=128,N=496 matmul) where fp32r was SBUF-BW bound
(~294 ns measured), and weight loads use FWL.

Per core: 7 ACT relus + 1 silu make the cube inputs per pixel, 2 DVE muls
cube them straight into the bf16 [pixel, krow] tile, PE-transpose to
[krow, pixel] (2 chunks per row share one PSUM tile -> single DVE cast),
then 9 taps x 2 K-chunks of bf16 matmuls accumulate over windows of 4 output
rows (N = 496 columns).
"""

import numpy as np
import ml_dtypes

KH = KW = 3
C = 32
FILTERS = 128
B, H, W = 16, 64, 64
OH = OW = 62
IN_SIZE = KH * KW * C  # 288
NCORES = 8
BLOC = B // NCORES  # 2 images per core

_NTAP = KH * KW  # 9
# feature class order (must match b4 column order and wpk krow order):
# chunk 0: [M3, M2, M1, R1]  chunk 1: [R2, R3, R4, silu]
# relu specs for the 7 cube features: v = relu(scale * x + bias)
_RELU_SPECS = [
    (-2.5, 0.5),   # M3 = relu(3 - t)
    (-2.5, -0.5),  # M2
    (-2.5, -1.5),  # M1
    (2.5, 1.5),    # R1 = relu(t - 1)
    (2.5, 0.5),    # R2
    (2.5, -0.5),   # R3
    (2.5, -1.5),   # R4
]

_program_cache = {}


def _prep_weights(spline_kernel, scale_factor, bias):
    """Returns (wpk [128, 18, 128] bf16, bias_eff [128, 1] fp32).

    wpk[krow, tap*2 + q, o]: krow = rloc*32 + c, class rc = q*4 + rloc in
    order [M3, M2, M1, R1, R2, R3, R4, silu].
    """
    sk = spline_kernel.astype(np.float64)  # (288, 8, 128)
    sf = scale_factor.astype(np.float64)  # (288, 128)
    w = sk * sf[:, None, :]  # (288, 8, 128)
    D = (w[:, 1:, :] - w[:, :-1, :]) / 6.0  # (288, 7, 128)

    W_M3 = -D[:, 2]
    W_M2 = -D[:, 1] + 3 * D[:, 2]
    W_M1 = -D[:, 0] + 3 * D[:, 1] - 3 * D[:, 2]
    W_R1 = D[:, 3]
    W_R2 = -3 * D[:, 3] + D[:, 4]
    W_R3 = 3 * D[:, 3] - 3 * D[:, 4] + D[:, 5]
    W_R4 = -D[:, 3] + 3 * D[:, 4] - 3 * D[:, 5] + D[:, 6]

    wfull = np.stack([W_M3, W_M2, W_M1, W_R1, W_R2, W_R3, W_R4, sf], axis=1)
    # (288, 8, 128) -> [tap, c, rc, o] -> [tap, rc, c, o]
    wfull = wfull.reshape(_NTAP, C, 8, FILTERS).transpose(0, 2, 1, 3)
    wpk = np.zeros((128, _NTAP * 2, FILTERS), dtype=np.float64)
    for tap in range(_NTAP):
        for rc in range(8):
            q, rloc = divmod(rc, 4)
            wpk[rloc * 32 : (rloc + 1) * 32, tap * 2 + q, :] = wfull[tap, rc]

    bias_eff = bias.astype(np.float64) + w[:, 3, :].sum(axis=0)
    return (
        wpk.astype(np.float32).astype(ml_dtypes.bfloat16),
        np.ascontiguousarray(bias_eff[:, None], dtype=np.float32),
    )


def _features_np(x):
    """Per-element features matching the device computation, already rounded
    to bf16.  x: (...,) -> (..., 8) in order [M3,M2,M1,R1,R2,R3,R4,silu]."""
    x = x.astype(np.float32)
    feats = []
    for sc, bc in _RELU_SPECS:
        v = np.maximum(np.float32(sc) * x + np.float32(bc), np.float32(0.0))
        feats.append((v * v) * v)
    sig = 1.0 / (1.0 + np.exp(-x.astype(np.float64)))
    feats.append((x.astype(np.float64) * sig).astype(np.float32))
    f = np.stack(feats, axis=-1)
    return f.astype(ml_dtypes.bfloat16).astype(np.float32)


def reference_sim(inputs, spline_kernel, scale_factor, bias, grid=None):
    """Host numpy simulation of the kernel math (for validation)."""
    wpk, bias_eff = _prep_weights(spline_kernel, scale_factor, bias)
    wpk = wpk.astype(np.float64)
    feats = _features_np(inputs)  # (B, H, W, 32, 8)
    out = np.zeros((inputs.shape[0], OH, OW, FILTERS), dtype=np.float64)
    for di in range(KH):
        for dj in range(KW):
            tap = di * 3 + dj
            f = feats[:, di : di + OH, dj : dj + OW]  # (B, OH, OW, 32, 8)
            for q in range(2):
                wq = wpk[:, tap * 2 + q, :]  # (128, 128)
                fq = f[..., :, q * 4 : (q + 1) * 4]  # (..., 32, 4) c, rloc
                fq = np.moveaxis(fq, -1, -2).reshape(*f.shape[:3], 128)
                out += fq @ wq
    return (out + bias_eff[:, 0]).astype(np.float32)


def _build_program():
    import concourse.mybir as mybir
    from concourse import bacc
    from concourse.tile import TileContext
    from concourse.masks import make_identity

    FP = mybir.dt.float32
    BF = mybir.dt.bfloat16
    AF = mybir.ActivationFunctionType

    nc = bacc.Bacc()
    # x is host-transposed to [img, x, row, c] so each DMA partition line
    # (one x column) reads a contiguous 4*32*4 = 512 B run per row-group
    x_d = nc.dram_tensor("x", [BLOC, W, H, C], FP, kind="ExternalInput")
    w_d = nc.dram_tensor("wpk", [128, _NTAP * 2, FILTERS], BF, kind="ExternalInput")
    b_d = nc.dram_tensor("bias_eff", [128, 1], FP, kind="ExternalInput")
    o_d = nc.dram_tensor("out", [128, OH, BLOC * OW], BF, kind="ExternalOutput")

    with TileContext(nc) as tc:
        with (
            tc.tile_pool(name="singles", bufs=1) as singles,
            tc.tile_pool(name="xp", bufs=3) as xp,
            tc.tile_pool(name="bp", bufs=2) as bp,
            tc.tile_pool(name="vp", bufs=2) as vp,
            tc.tile_pool(name="op", bufs=2) as op,
            tc.tile_pool(name="pt", bufs=4, space="PSUM") as pt,
            tc.tile_pool(name="po", bufs=2, space="PSUM") as po,
            tc.tile_pool(name="pw", bufs=1, space="PSUM") as pw,
        ):
            # group-0 x loads go first so the sync DMA queue starts them
            # during boot
            x4_0 = xp.tile([128, 4, C], FP, name="x4_0", tag="x4")
            for im in range(BLOC):
                src0 = x_d[im, :, 0:4, :]
                nc.sync.dma_start(out=x4_0[im * 64 : (im + 1) * 64, :, :], in_=src0)

            # identity first on gpsimd (junk matmuls need it early)
            identb = singles.tile([128, 128], BF)
            make_identity(nc, identb)
            # relu biases: columns [1.5, 0.5, -0.5, -1.5]
            rbias = singles.tile([128, 4], FP)
            for i, val in enumerate((1.5, 0.5, -0.5, -1.5)):
                nc.gpsimd.memset(rbias[:, i : i + 1], val)
            wt = singles.tile([128, _NTAP * 2, FILTERS], BF)
            biasT = singles.tile([128, 1], FP)
            # weight chunks split across all three DGE queues right at boot
            # (294 KB is ~13us on one queue); the scalar-queue issue comes
            # before any ACT work so its transfer starts immediately
            w_engs = [nc.gpsimd, nc.gpsimd, nc.scalar, nc.sync, nc.sync, nc.sync]
            for wch in range(6):
                w_engs[wch].dma_start(
                    out=wt[:, wch * 3 : (wch + 1) * 3, :],
                    in_=w_d[:, wch * 3 : (wch + 1) * 3, :],
                )
            nc.gpsimd.dma_start(out=biasT, in_=b_d[:, :])
            # pre-warm only the Relu ACT table (on the group-0 critical path);
            # the Silu table load is hidden inside phase_a(0)
            warm = singles.tile([128, 1], FP)
            nc.scalar.activation(warm, rbias[:, 0:1], AF.Relu, bias=rbias[:, 0:1], scale=1.0)
            # ~1.7us of junk matmuls during boot: HAM only counts real MMs as
            # PE-busy (not transposes), so this un-throttles the clock before
            # the first real work instead of ~8us into it
            junkps = pw.tile([128, 128], FP)
            for _ in range(16):
                nc.tensor.matmul(junkps, identb, identb, start=True, stop=True)
            # transposed features: [krow 128, chunk q 2, row 64, img 2, x 64]
            btAll = singles.tile([128, 2, H, BLOC, 64], BF)

            # bias column index for each of the 7 relu specs
            bmap = []
            bvals = (1.5, 0.5, -0.5, -1.5)
            for _sc, bc in _RELU_SPECS:
                bmap.append(bvals.index(bc))

            # Phase A (per group of 4 input rows): 7 relu cubes + silu in
            # [pixel, class*32+c] layout, PE-transpose into btAll.
            def phase_a(g):
                if g == 0:
                    x4 = x4_0
                else:
                    x4 = xp.tile([128, 4, C], FP, name=f"x4_{g}", tag="x4")
                    for im in range(BLOC):
                        src = x_d[im, :, g * 4 : (g + 1) * 4, :]
                        nc.sync.dma_start(
                            out=x4[im * 64 : (im + 1) * 64, :, :], in_=src
                        )
                b4 = bp.tile([128, 4, 256], BF, name=f"b4_{g}", tag="b4")
                V = vp.tile([128, 4, 224], FP, name=f"V_{g}", tag="V")
                V2 = vp.tile([128, 4, 224], FP, name=f"V2_{g}", tag="V2")

                for idx, (sc, _bc) in enumerate(_RELU_SPECS):
                    nc.scalar.activation(
                        V[:, :, idx * 32 : (idx + 1) * 32],
                        x4,
                        AF.Relu,
                        bias=rbias[:, bmap[idx] : bmap[idx] + 1],
                        scale=sc,
                    )
                    if g == 0 and idx == 3:
                        # Silu table load (~1.3us) hides behind chunk-0
                        # cube/transpose work instead of gating it
                        nc.scalar.activation(warm, rbias[:, 0:1], AF.Silu)
                nc.scalar.activation(b4[:, :, 224:256], x4, AF.Silu)
                # split squares/cubes per chunk so chunk-0 transposes don't
                # wait on chunk-1 relus
                nc.vector.tensor_mul(V2[:, :, 0:128], V[:, :, 0:128], V[:, :, 0:128])
                nc.vector.tensor_mul(b4[:, :, 0:128], V2[:, :, 0:128], V[:, :, 0:128])
                nc.vector.tensor_mul(V2[:, :, 128:224], V[:, :, 128:224], V[:, :, 128:224])
                nc.vector.tensor_mul(b4[:, :, 128:224], V2[:, :, 128:224], V[:, :, 128:224])

                for r in range(4):
                    row = g * 4 + r
                    ptile = pt.tile([128, 2, 128], BF, name=f"pt_{g}_{r}", tag="pt")
                    for q in range(2):
                        nc.tensor.transpose(
                            ptile[:, q, :], b4[:, r, q * 128 : (q + 1) * 128], identb
                        )
                    # psum free order (q, img*64+x) == btAll (q, img, x)
                    src = ptile.rearrange("p q (i x) -> p q i x", i=BLOC)
                    nc.vector.tensor_copy(btAll[:, :, row], src)

            # Phase B (per group of output rows; groups 0-14 are 4 rows with
            # N = 496 columns, group 15 is the final 2 rows, N = 248).
            # Out-DMAs rotate across engine DGE queues so the store stream
            # doesn't serialize on one queue and drag out the kernel tail.
            # sync is reserved for the x-load stream; stores go on the other two
            dma_engines = [nc.scalar, nc.gpsimd]

            def phase_b(og):
                y0 = og * 4
                nrow = 4 if og < 15 else 2
                ps = po.tile([128, 4, 124], FP, name=f"ps_{og}", tag="ps")
                idx = 0
                for di in range(KH):
                    for dj in range(KW):
                        for q in range(2):
                            rhs = btAll[:, q, y0 + di : y0 + di + nrow, :, dj : dj + 62]
                            nc.tensor.matmul(
                                ps[:, 0:nrow, :],
                                wt[:, (di * 3 + dj) * 2 + q, :],
                                rhs,
                                start=(idx == 0),
                                stop=(idx == 17),
                            )
                            idx += 1
                ot = op.tile([128, 4, 124], BF, name=f"ot_{og}", tag="ot")
                nc.scalar.activation(
                    ot[:, 0:nrow, :],
                    ps[:, 0:nrow, :],
                    AF.Identity,
                    bias=biasT[:, 0:1],
                    scale=1.0,
                )
                # two store pieces on different DGE queues -> two DMA engines
                # run them concurrently (one dma_start = one engine)
                half = nrow // 2
                for piece in range(2):
                    r0 = piece * half
                    dma_engines[piece].dma_start(
                        out=o_d[:, y0 + r0 : y0 + r0 + half, :],
                        in_=ot[:, r0 : r0 + half, :],
                    )

            # Interleave phase_b(og) after phase_a(og + 1); weight load
            # chunked on the gpsimd SWDGE queue after group 0's x loads.
            for g in range(H // 4):
                phase_a(g)
                if g >= 1:
                    phase_b(g - 1)
            phase_b(15)
    nc.compile()
    return nc


def _get_program():
    if "nc" not in _program_cache:
        _program_cache["nc"] = _build_program()
    return _program_cache["nc"]


def run_cores(inputs, spline_kernel, scale_factor, bias, trace=False):
    """Run the SPMD kernel on 8 cores; returns (out, BassKernelResults)."""
    from concourse.bass_utils import run_bass_kernel_spmd

    wpk, bias_eff = _prep_weights(spline_kernel, scale_factor, bias)
    # host-side transpose to [img, x(W), row(H), c] for contiguous DMA runs
    x = np.ascontiguousarray(
        np.transpose(inputs.astype(np.float32), (0, 2, 1, 3))
    )
    in_maps = [
        {
            "x": x[i * BLOC : (i + 1) * BLOC],
            "wpk": wpk,
            "bias_eff": bias_eff,
        }
        for i in range(NCORES)
    ]
    nc = _get_program()
    res = run_bass_kernel_spmd(nc, in_maps, list(range(NCORES)), trace=trace)
    out = np.empty((B, OH, OW, FILTERS), dtype=np.float32)
    for i in range(NCORES):
        oc = np.asarray(res.results[i]["out"], dtype=np.float32)
        oc = oc.reshape(128, OH, BLOC, OW)
        out[i * BLOC : (i + 1) * BLOC] = np.transpose(oc, (2, 1, 3, 0))
    return out, res


def kernel(inputs, spline_kernel, scale_factor, bias, grid=None, **_):
    out, _res = run_cores(inputs, spline_kernel, scale_factor, bias, trace=False)
    return out
